# revision 1
# baseline (speedup 1.0000x reference)
"""MoE layer (top-2 of 24 experts, d_model=1024, d_ff=4096, T=4096 tokens)
on 8 Trainium2 NeuronCores.

Strategy (expert-parallel, host-routed):
  - Host computes the gate (x @ gate_w + gate_b), top-2 expert ids and
    softmax probs, then gathers each expert's tokens into a padded,
    transposed buffer xT[e] = [D, C] (C = max tokens/expert, 128-aligned).
  - Experts are sharded 3 per core. Each core runs a Bass/Tile kernel:
      hT = gelu(w1[e].T-tiled @ xT[e] + b1[e])   (PE + ACT, bf16 in / fp32 acc)
      y  = prob * (hT.T-tiled @ w2[e])           (PE + DVE scale)
  - Host scatters the two per-token expert outputs back together
    (out[t] = y[slot0(t)] + y[slot1(t)]), adds the b2 combine term if b2
    is nonzero (it is zero in this problem's setup_inputs).

Matmuls run in bf16 with fp32 PSUM accumulation (rel err ~3e-3 vs fp32);
b1 is applied exactly on device as the ACT per-partition bias.
"""

import numpy as np
import ml_dtypes

P = 128
D_MODEL = 1024
D_FF = 4096
NUM_EXPERTS = 24
TOP_K = 2
N_CORES = 8
E_LOC = NUM_EXPERTS // N_CORES   # 3 experts per core
KD = D_MODEL // P                # 8  k-chunks over d_model
KF = D_FF // P                   # 32 k-chunks over d_ff
ND = D_MODEL // 512              # 2  512-wide output chunks
BF16 = ml_dtypes.bfloat16


def _build(C):
    """Build the per-core Bass program (SPMD: same program, per-core data).

    C: per-expert token capacity (multiple of 128).
    """
    import concourse.bacc as bacc
    import concourse.mybir as mybir
    from concourse.tile import TileContext

    dt = mybir.dt.bfloat16
    f32 = mybir.dt.float32
    NTC = C // P                 # 128-token subtiles per expert

    nc = bacc.Bacc(None, target_bir_lowering=False)
    xT = nc.dram_tensor("xt", [E_LOC, KD, P, C], dt, kind="ExternalInput")
    w1 = nc.dram_tensor("w1", [E_LOC, KD, P, D_FF], dt, kind="ExternalInput")
    w2 = nc.dram_tensor("w2", [E_LOC, KF, P, D_MODEL], dt, kind="ExternalInput")
    pr = nc.dram_tensor("pr", [P, E_LOC * NTC], f32, kind="ExternalInput")
    b1 = nc.dram_tensor("b1", [P, E_LOC * KF], f32, kind="ExternalInput")
    y = nc.dram_tensor("y", [E_LOC, C, D_MODEL], f32, kind="ExternalOutput")

    with TileContext(nc) as tc:
        with tc.tile_pool(name="consts", bufs=1) as consts, \
             tc.tile_pool(name="xtp", bufs=E_LOC * KD) as xtp, \
             tc.tile_pool(name="w1p", bufs=KD + 2) as w1p, \
             tc.tile_pool(name="w2p", bufs=40) as w2p, \
             tc.tile_pool(name="htp", bufs=KF + 2) as htp, \
             tc.tile_pool(name="outp", bufs=4) as outp, \
             tc.tile_pool(name="psA", bufs=3, space="PSUM") as psA, \
             tc.tile_pool(name="psB", bufs=3, space="PSUM") as psB:

            pr_t = consts.tile([P, E_LOC * NTC], f32, tag="pr")
            nc.sync.dma_start(pr_t[:], pr[:, :])
            b1_t = consts.tile([P, E_LOC * KF], f32, tag="b1")
            nc.sync.dma_start(b1_t[:], b1[:, :])

            xts = {}
            for e in range(E_LOC):
                for k in range(KD):
                    t_ = xtp.tile([P, C], dt, tag="xt")
                    nc.sync.dma_start(t_[:], xT[e, k])
                    xts[(e, k)] = t_

            for e in range(E_LOC):
                w1ts = []
                for k in range(KD):
                    t_ = w1p.tile([P, D_FF], dt, tag="w1")
                    nc.sync.dma_start(t_[:], w1[e, k])
                    w1ts.append(t_)

                NT = (C + 511) // 512
                for t in range(NT):
                    c0 = t * 512
                    W = min(512, C - c0)
                    # phase A: hT[m] = gelu(w1k.T @ xT + b1), [P dff x W tok]
                    hts = []
                    for m in range(KF):
                        ps = psA.tile([P, 512], f32, tag="psA")
                        for k in range(KD):
                            nc.tensor.matmul(
                                ps[:, :W],
                                w1ts[k][:, m * P:(m + 1) * P],
                                xts[(e, k)][:, c0:c0 + W],
                                start=(k == 0), stop=(k == KD - 1))
                        ht = htp.tile([P, 512], dt, tag="ht")
                        nc.scalar.activation(
                            ht[:, :W], ps[:, :W],
                            mybir.ActivationFunctionType.Gelu,
                            bias=b1_t[:, e * KF + m: e * KF + m + 1])
                        hts.append(ht)
                    # phase B: y[s-tile] = prob * (hT.T @ w2), [P tok x 512 dm]
                    for n in range(ND):
                        w2ts = []
                        for k in range(KF):
                            t_ = w2p.tile([P, 512], dt, tag="w2")
                            nc.sync.dma_start(
                                t_[:], w2[e, k, :, n * 512:(n + 1) * 512])
                            w2ts.append(t_)
                        for s in range(W // P):
                            ps = psB.tile([P, 512], f32, tag="psB")
                            for k in range(KF):
                                nc.tensor.matmul(
                                    ps[:], hts[k][:, s * P:(s + 1) * P],
                                    w2ts[k][:],
                                    start=(k == 0), stop=(k == KF - 1))
                            ot = outp.tile([P, 512], f32, tag="out")
                            col = e * NTC + c0 // P + s
                            nc.vector.tensor_scalar_mul(
                                ot[:], ps[:], pr_t[:, col:col + 1])
                            nc.sync.dma_start(
                                y[e, c0 + s * P: c0 + (s + 1) * P,
                                  n * 512:(n + 1) * 512], ot[:])
    nc.finalize()
    return nc


def _route(x, gate_w, gate_b):
    """Top-2 routing on host. Returns per-(token,k) expert/prob flattened,
    sorted-by-expert order, per-expert counts, and the y-row index for each
    routed pair."""
    T = x.shape[0]
    scores = x @ gate_w + gate_b                      # [T, E]
    part = np.argpartition(scores, -TOP_K, axis=1)[:, -TOP_K:]   # [T, 2]
    vals = np.take_along_axis(scores, part, axis=1)
    vmax = vals.max(axis=1, keepdims=True)
    ex = np.exp(vals - vmax)
    prob = ex / ex.sum(axis=1, keepdims=True)

    expert_flat = part.ravel()                        # [2T]
    prob_flat = prob.ravel().astype(np.float32)
    token_flat = np.repeat(np.arange(T), TOP_K)

    order = np.argsort(expert_flat, kind="stable")
    counts = np.bincount(expert_flat, minlength=NUM_EXPERTS)
    starts = np.zeros(NUM_EXPERTS + 1, dtype=np.int64)
    np.cumsum(counts, out=starts[1:])

    inv_order = np.empty_like(order)
    inv_order[order] = np.arange(order.size)
    # position of pair p within its expert segment
    pos = inv_order - starts[expert_flat]
    return (expert_flat, prob_flat, token_flat, order, counts, starts, pos)


def kernel(x, gate_w, gate_b, w1, b1, w2, b2):
    from concourse import bass_utils

    B, S, D = x.shape
    T = B * S
    xf = np.ascontiguousarray(x.reshape(T, D), dtype=np.float32)

    (expert_flat, prob_flat, token_flat, order, counts, starts, pos) = _route(
        xf, np.asarray(gate_w, np.float32), np.asarray(gate_b, np.float32))

    C = max(P, int(-(-counts.max() // P)) * P)        # capacity, 128-aligned

    # gather tokens by expert, transposed + bf16, padded to C
    xg16 = xf[token_flat[order]].astype(BF16)         # [2T, D] sorted by expert
    xT_g = np.zeros((NUM_EXPERTS, D, C), dtype=BF16)
    pr_g = np.zeros((NUM_EXPERTS, C), dtype=np.float32)
    sorted_probs = prob_flat[order]
    for e in range(NUM_EXPERTS):
        c_e = counts[e]
        if c_e:
            seg = slice(starts[e], starts[e] + c_e)
            xT_g[e, :, :c_e] = xg16[seg].T
            pr_g[e, :c_e] = sorted_probs[seg]

    w1_16 = np.asarray(w1, np.float32).astype(BF16)   # [E, D, F]
    w2_16 = np.asarray(w2, np.float32).astype(BF16)   # [E, F, D]
    b1_f = np.asarray(b1, np.float32)                 # [E, F]

    NTC = C // P
    in_maps = []
    for c in range(N_CORES):
        sl = slice(c * E_LOC, (c + 1) * E_LOC)
        in_maps.append({
            "xt": np.ascontiguousarray(
                xT_g[sl].reshape(E_LOC, KD, P, C)),
            "w1": np.ascontiguousarray(
                w1_16[sl].reshape(E_LOC, KD, P, D_FF)),
            "w2": np.ascontiguousarray(
                w2_16[sl].reshape(E_LOC, KF, P, D_MODEL)),
            "pr": np.ascontiguousarray(
                pr_g[sl].reshape(E_LOC, NTC, P).transpose(2, 0, 1)
                .reshape(P, E_LOC * NTC)),
            "b1": np.ascontiguousarray(
                b1_f[sl].reshape(E_LOC, KF, P).transpose(2, 0, 1)
                .reshape(P, E_LOC * KF)),
        })

    nc = _build(C)
    res = bass_utils.run_bass_kernel_spmd(nc, in_maps, core_ids=list(range(N_CORES)))

    y_all = np.concatenate([res.results[c]["y"] for c in range(N_CORES)], axis=0)
    y_flat = y_all.reshape(NUM_EXPERTS * C, D)

    # combine: out[t] = sum over the token's two routed slots
    rows = expert_flat.astype(np.int64) * C + pos     # [2T] y-row per pair
    rows = rows.reshape(T, TOP_K)
    out = y_flat[rows[:, 0]] + y_flat[rows[:, 1]]

    b2_f = np.asarray(b2, np.float32)
    if np.any(b2_f):
        combine = np.zeros((T, NUM_EXPERTS), dtype=np.float32)
        np.add.at(combine, (token_flat, expert_flat), prob_flat)
        out += combine @ b2_f

    return np.ascontiguousarray(out.reshape(B, S, D), dtype=np.float32)


# revision 4
# speedup vs baseline: 250.2367x; 250.2367x over previous
"""MoE layer (top-2 of 24 experts, d_model=1024, d_ff=4096, T=4096 tokens)
on 8 Trainium2 NeuronCores.

Strategy (expert-parallel, host-routed):
  - Host computes the gate (x @ gate_w + gate_b), top-2 expert ids and
    softmax probs, then gathers each expert's tokens into a padded,
    transposed buffer xT[e] = [D, C_j] (capacity 128-aligned).
  - Experts are sharded 3 per core, balanced by token count: experts are
    sorted by count and dealt into 3 "slots" (slot j of every core holds
    one of the j-th-octile experts), so slot capacities C_0 >= C_1 >= C_2
    track the count distribution instead of all padding to the global max.
  - Each core runs a Bass/Tile kernel per slot expert:
      hT = gelu(w1[e].T-tiled @ xT[e] + b1[e])   (PE + ACT, bf16 in / fp32 acc)
      y  = prob * (hT.T-tiled @ w2[e])           (PE + DVE scale)
  - Host scatters the two per-token expert outputs back together
    (out[t] = y[slot0(t)] + y[slot1(t)]), adds the b2 combine term if b2
    is nonzero (it is zero in this problem's setup_inputs).

Matmuls run in bf16 with fp32 PSUM accumulation (rel err ~3e-3 vs fp32);
b1 is applied exactly on device as the ACT per-partition bias.
"""

import numpy as np
import ml_dtypes

P = 128
D_MODEL = 1024
D_FF = 4096
NUM_EXPERTS = 24
TOP_K = 2
N_CORES = 8
E_LOC = NUM_EXPERTS // N_CORES   # 3 experts per core
KD = D_MODEL // P                # 8  k-chunks over d_model
KF = D_FF // P                   # 32 k-chunks over d_ff
ND = D_MODEL // 512              # 2  512-wide output chunks
BF16 = ml_dtypes.bfloat16


def _build(Cs, repeat=1):
    """Build the per-core Bass program (SPMD: same program, per-core data).

    Cs: per-slot token capacities (each a multiple of 128).
    repeat: run the whole compute N times (timing rigs only).
    """
    import concourse.bacc as bacc
    import concourse.mybir as mybir
    from concourse.tile import TileContext

    dt = mybir.dt.bfloat16
    f32 = mybir.dt.float32
    CT = sum(Cs)
    offs = [sum(Cs[:j]) for j in range(E_LOC)]
    NTC = CT // P                # 128-token subtiles across all slots

    nc = bacc.Bacc(None, target_bir_lowering=False)
    xT = nc.dram_tensor("xt", [KD, P, CT], dt, kind="ExternalInput")
    w1 = nc.dram_tensor("w1", [E_LOC, KD, P, D_FF], dt, kind="ExternalInput")
    w2 = nc.dram_tensor("w2", [E_LOC, KF, P, D_MODEL], dt, kind="ExternalInput")
    pr = nc.dram_tensor("pr", [P, NTC], f32, kind="ExternalInput")
    b1 = nc.dram_tensor("b1", [P, E_LOC * KF], f32, kind="ExternalInput")
    y = nc.dram_tensor("y", [CT, D_MODEL], f32, kind="ExternalOutput")

    with TileContext(nc) as tc:
        with tc.tile_pool(name="consts", bufs=1) as consts, \
             tc.tile_pool(name="xtp", bufs=KD) as xtp, \
             tc.tile_pool(name="w1p", bufs=KD + 2) as w1p, \
             tc.tile_pool(name="w2p", bufs=40) as w2p, \
             tc.tile_pool(name="htp", bufs=KF + 2) as htp, \
             tc.tile_pool(name="outp", bufs=4) as outp, \
             tc.tile_pool(name="psA", bufs=4, space="PSUM") as psA, \
             tc.tile_pool(name="psB", bufs=4, space="PSUM") as psB:

            pr_t = consts.tile([P, NTC], f32, tag="pr")
            nc.sync.dma_start(pr_t[:], pr[:, :])
            b1_t = consts.tile([P, E_LOC * KF], f32, tag="b1")
            nc.sync.dma_start(b1_t[:], b1[:, :])

            xts = []
            for k in range(KD):
                t_ = xtp.tile([P, CT], dt, tag="xt")
                nc.sync.dma_start(t_[:], xT[k])
                xts.append(t_)

            for e in [e for _ in range(repeat) for e in range(E_LOC)]:
                C = Cs[e]
                off = offs[e]
                w1ts = []
                for k in range(KD):
                    t_ = w1p.tile([P, D_FF], dt, tag="w1")
                    nc.sync.dma_start(t_[:], w1[e, k])
                    w1ts.append(t_)

                NT = (C + 511) // 512
                for t in range(NT):
                    c0 = t * 512
                    W = min(512, C - c0)
                    # phase A: hT[m] = gelu(w1k.T @ xT + b1), [P dff x W tok]
                    hts = []
                    for m in range(KF):
                        ps = psA.tile([P, 512], f32, tag="psA")
                        for k in range(KD):
                            nc.tensor.matmul(
                                ps[:, :W],
                                w1ts[k][:, m * P:(m + 1) * P],
                                xts[k][:, off + c0:off + c0 + W],
                                start=(k == 0), stop=(k == KD - 1))
                        ht = htp.tile([P, 512], dt, tag="ht")
                        nc.scalar.activation(
                            ht[:, :W], ps[:, :W],
                            mybir.ActivationFunctionType.Gelu,
                            bias=b1_t[:, e * KF + m: e * KF + m + 1])
                        hts.append(ht)
                    # phase B: y[s-tile] = prob * (hT.T @ w2), [P tok x 512 dm]
                    for n in range(ND):
                        w2ts = []
                        for k in range(KF):
                            t_ = w2p.tile([P, 512], dt, tag="w2")
                            nc.sync.dma_start(
                                t_[:], w2[e, k, :, n * 512:(n + 1) * 512])
                            w2ts.append(t_)
                        for s in range(W // P):
                            ps = psB.tile([P, 512], f32, tag="psB")
                            for k in range(KF):
                                nc.tensor.matmul(
                                    ps[:], hts[k][:, s * P:(s + 1) * P],
                                    w2ts[k][:],
                                    start=(k == 0), stop=(k == KF - 1))
                            ot = outp.tile([P, 512], f32, tag="out")
                            col = (off + c0) // P + s
                            nc.vector.tensor_scalar_mul(
                                ot[:], ps[:], pr_t[:, col:col + 1])
                            nc.sync.dma_start(
                                y[off + c0 + s * P: off + c0 + (s + 1) * P,
                                  n * 512:(n + 1) * 512], ot[:])
    nc.finalize()
    return nc


def _route(x, gate_w, gate_b):
    """Top-2 routing on host. Returns flattened (expert, prob) per routed
    pair, the by-expert sort order, per-expert counts/starts, and each
    pair's position within its expert segment."""
    T = x.shape[0]
    scores = x @ gate_w + gate_b                      # [T, E]
    part = np.argpartition(scores, -TOP_K, axis=1)[:, -TOP_K:]   # [T, 2]
    vals = np.take_along_axis(scores, part, axis=1)
    vmax = vals.max(axis=1, keepdims=True)
    ex = np.exp(vals - vmax)
    prob = ex / ex.sum(axis=1, keepdims=True)

    expert_flat = part.ravel()                        # [2T]
    prob_flat = prob.ravel().astype(np.float32)
    token_flat = np.repeat(np.arange(T), TOP_K)

    order = np.argsort(expert_flat, kind="stable")
    counts = np.bincount(expert_flat, minlength=NUM_EXPERTS)
    starts = np.zeros(NUM_EXPERTS + 1, dtype=np.int64)
    np.cumsum(counts, out=starts[1:])

    inv_order = np.empty_like(order)
    inv_order[order] = np.arange(order.size)
    pos = inv_order - starts[expert_flat]
    return (expert_flat, prob_flat, token_flat, order, counts, starts, pos)


def _prepare(x, gate_w, gate_b, w1, b1, w2, b2):
    """Host-side routing, balanced expert->(core,slot) assignment, and
    per-core input packing. Returns (in_maps, Cs, meta-for-combine)."""
    B, S, D = x.shape
    T = B * S
    xf = np.ascontiguousarray(x.reshape(T, D), dtype=np.float32)

    (expert_flat, prob_flat, token_flat, order, counts, starts, pos) = _route(
        xf, np.asarray(gate_w, np.float32), np.asarray(gate_b, np.float32))

    # balanced assignment: slot j of core c holds expert_desc[j*8 + c]
    expert_desc = np.argsort(-counts, kind="stable")
    core_of = np.empty(NUM_EXPERTS, dtype=np.int64)
    slot_of = np.empty(NUM_EXPERTS, dtype=np.int64)
    for j in range(E_LOC):
        for c in range(N_CORES):
            e = expert_desc[j * N_CORES + c]
            core_of[e] = c
            slot_of[e] = j
    Cs = []
    for j in range(E_LOC):
        mx = counts[expert_desc[j * N_CORES:(j + 1) * N_CORES]].max()
        Cs.append(max(P, int(-(-int(mx) // P)) * P))
    CT = sum(Cs)
    offs = [sum(Cs[:j]) for j in range(E_LOC)]

    xg16 = xf[token_flat[order]].astype(BF16)         # [2T, D] sorted by expert
    sorted_probs = prob_flat[order]

    w1_16 = np.asarray(w1, np.float32).astype(BF16)   # [E, D, F]
    w2_16 = np.asarray(w2, np.float32).astype(BF16)   # [E, F, D]
    b1_f = np.asarray(b1, np.float32)                 # [E, F]

    NTC = CT // P
    in_maps = []
    for c in range(N_CORES):
        xt_core = np.zeros((D, CT), dtype=BF16)
        pr_core = np.zeros((CT,), dtype=np.float32)
        w1_core = np.empty((E_LOC, D, D_FF), dtype=BF16)
        w2_core = np.empty((E_LOC, D_FF, D), dtype=BF16)
        b1_core = np.empty((E_LOC, D_FF), dtype=np.float32)
        for j in range(E_LOC):
            e = expert_desc[j * N_CORES + c]
            c_e = counts[e]
            if c_e:
                seg = slice(starts[e], starts[e] + c_e)
                xt_core[:, offs[j]:offs[j] + c_e] = xg16[seg].T
                pr_core[offs[j]:offs[j] + c_e] = sorted_probs[seg]
            w1_core[j] = w1_16[e]
            w2_core[j] = w2_16[e]
            b1_core[j] = b1_f[e]
        in_maps.append({
            "xt": np.ascontiguousarray(xt_core.reshape(KD, P, CT)),
            "w1": np.ascontiguousarray(w1_core.reshape(E_LOC, KD, P, D_FF)),
            "w2": np.ascontiguousarray(w2_core.reshape(E_LOC, KF, P, D_MODEL)),
            "pr": np.ascontiguousarray(
                pr_core.reshape(NTC, P).T),
            "b1": np.ascontiguousarray(
                b1_core.reshape(E_LOC, KF, P).transpose(2, 0, 1)
                .reshape(P, E_LOC * KF)),
        })

    meta = dict(T=T, shape=x.shape, CT=CT, offs=offs,
                core_of=core_of, slot_of=slot_of,
                expert_flat=expert_flat, prob_flat=prob_flat,
                token_flat=token_flat, pos=pos, b2=np.asarray(b2, np.float32))
    return in_maps, Cs, meta


def _combine(y_per_core, meta):
    """out[t] = sum of the token's two routed expert outputs (+ b2 term)."""
    T = meta["T"]
    CT = meta["CT"]
    offs = np.asarray(meta["offs"], dtype=np.int64)
    expert_flat = meta["expert_flat"]
    y_flat = np.concatenate(y_per_core, axis=0)       # [8*CT, D]

    rows = (meta["core_of"][expert_flat] * CT
            + offs[meta["slot_of"][expert_flat]] + meta["pos"])
    rows = rows.reshape(T, TOP_K)
    out = y_flat[rows[:, 0]] + y_flat[rows[:, 1]]

    b2_f = meta["b2"]
    if np.any(b2_f):
        combine = np.zeros((T, NUM_EXPERTS), dtype=np.float32)
        np.add.at(combine, (meta["token_flat"], expert_flat), meta["prob_flat"])
        out += combine @ b2_f
    return np.ascontiguousarray(out.reshape(meta["shape"]), dtype=np.float32)


def kernel(x, gate_w, gate_b, w1, b1, w2, b2):
    from concourse import bass_utils

    in_maps, Cs, meta = _prepare(x, gate_w, gate_b, w1, b1, w2, b2)
    nc = _build(Cs)
    res = bass_utils.run_bass_kernel_spmd(nc, in_maps, core_ids=list(range(N_CORES)))
    return _combine([res.results[c]["y"] for c in range(N_CORES)], meta)


# revision 5
# speedup vs baseline: 293.6803x; 1.1736x over previous
"""MoE layer (top-2 of 24 experts, d_model=1024, d_ff=4096, T=4096 tokens)
on 8 Trainium2 NeuronCores.

Strategy (expert-parallel, host-routed):
  - Host computes the gate (x @ gate_w + gate_b), top-2 expert ids and
    softmax probs, then gathers each expert's tokens into a padded,
    transposed buffer xT[e] = [D, C_j] (capacity 128-aligned).
  - Experts are sharded 3 per core, balanced by token count: experts are
    sorted by count and dealt into 3 "slots" (slot j of every core holds
    one of the j-th-octile experts), so slot capacities C_0 >= C_1 >= C_2
    track the count distribution instead of all padding to the global max.
  - Each core runs a Bass/Tile kernel per slot expert:
      hT = gelu(w1[e].T-tiled @ xT[e] + b1[e])   (PE + ACT, bf16 in / fp32 acc)
      y  = prob * (hT.T-tiled @ w2[e])           (PE + DVE scale)
  - Host scatters the two per-token expert outputs back together
    (out[t] = y[slot0(t)] + y[slot1(t)]), adds the b2 combine term if b2
    is nonzero (it is zero in this problem's setup_inputs).

Matmuls run in bf16 with fp32 PSUM accumulation (rel err ~3e-3 vs fp32);
b1 is applied exactly on device as the ACT per-partition bias.
"""

import numpy as np
import ml_dtypes

P = 128
D_MODEL = 1024
D_FF = 4096
NUM_EXPERTS = 24
TOP_K = 2
N_CORES = 8
E_LOC = NUM_EXPERTS // N_CORES   # 3 experts per core
KD = D_MODEL // P                # 8  k-chunks over d_model
KF = D_FF // P                   # 32 k-chunks over d_ff
ND = D_MODEL // 512              # 2  512-wide output chunks
BF16 = ml_dtypes.bfloat16


def _build(Cs, repeat=1):
    """Build the per-core Bass program (SPMD: same program, per-core data).

    Cs: per-slot token capacities (each a multiple of 128).
    repeat: run the whole compute N times (timing rigs only).
    """
    import concourse.bacc as bacc
    import concourse.mybir as mybir
    from concourse.tile import TileContext

    dt = mybir.dt.bfloat16
    f32 = mybir.dt.float32
    CT = sum(Cs)
    offs = [sum(Cs[:j]) for j in range(E_LOC)]
    NTC = CT // P                # 128-token subtiles across all slots

    nc = bacc.Bacc(None, target_bir_lowering=False)
    xT = nc.dram_tensor("xt", [KD, P, CT], dt, kind="ExternalInput")
    w1 = nc.dram_tensor("w1", [E_LOC, KD, P, D_FF], dt, kind="ExternalInput")
    w2 = nc.dram_tensor("w2", [E_LOC, KF, P, D_MODEL], dt, kind="ExternalInput")
    pr = nc.dram_tensor("pr", [P, NTC], f32, kind="ExternalInput")
    b1 = nc.dram_tensor("b1", [P, E_LOC * KF], f32, kind="ExternalInput")
    y = nc.dram_tensor("y", [CT, D_MODEL], f32, kind="ExternalOutput")

    W1C = 4                      # w1 k-tiles DMA'd in 4 column chunks
    W1CW = D_FF // W1C           # 1024 columns per chunk
    MPC = W1CW // P              # 8 m-tiles per chunk

    with TileContext(nc) as tc:
        with tc.tile_pool(name="consts", bufs=1) as consts, \
             tc.tile_pool(name="xtp", bufs=E_LOC * KD) as xtp, \
             tc.tile_pool(name="w1p", bufs=(W1C * KD) + 8) as w1p, \
             tc.tile_pool(name="w2p", bufs=40) as w2p, \
             tc.tile_pool(name="htp", bufs=KF + 2) as htp, \
             tc.tile_pool(name="outp", bufs=4) as outp, \
             tc.tile_pool(name="psA", bufs=4, space="PSUM") as psA, \
             tc.tile_pool(name="psB", bufs=4, space="PSUM") as psB:

            b1_t = consts.tile([P, E_LOC * KF], f32, tag="b1")
            nc.sync.dma_start(b1_t[:], b1[:, :])
            pr_t = consts.tile([P, NTC], f32, tag="pr")
            nc.sync.dma_start(pr_t[:], pr[:, :])

            xts = {}   # (slot j, k) -> [P, Cs[j]] tile
            w1ts = {}  # (k, chunk) -> [P, 1024] tile, current expert

            def load_w1_chunk(e, c4):
                for k in range(KD):
                    t_ = w1p.tile([P, W1CW], dt, tag="w1")
                    nc.sync.dma_start(
                        t_[:], w1[e, k, :, c4 * W1CW:(c4 + 1) * W1CW])
                    w1ts[(k, c4)] = t_

            def load_xt(j):
                for k in range(KD):
                    t_ = xtp.tile([P, Cs[j]], dt, tag="xt")
                    nc.sync.dma_start(
                        t_[:], xT[k, :, offs[j]:offs[j] + Cs[j]])
                    xts[(j, k)] = t_

            for e in [e for _ in range(repeat) for e in range(E_LOC)]:
                C = Cs[e]
                off = offs[e]
                load_w1_chunk(e, 0)
                if (e, 0) not in xts:
                    load_xt(e)
                for c4 in range(1, W1C):
                    load_w1_chunk(e, c4)

                NT = (C + 511) // 512
                for t in range(NT):
                    c0 = t * 512
                    W = min(512, C - c0)
                    # phase A: hT[m] = gelu(w1k.T @ xT + b1), [P dff x W tok]
                    hts = []
                    for m in range(KF):
                        ps = psA.tile([P, 512], f32, tag="psA")
                        for k in range(KD):
                            nc.tensor.matmul(
                                ps[:, :W],
                                w1ts[(k, m // MPC)][:, (m % MPC) * P:
                                                    (m % MPC + 1) * P],
                                xts[(e, k)][:, c0:c0 + W],
                                start=(k == 0), stop=(k == KD - 1))
                        ht = htp.tile([P, 512], dt, tag="ht")
                        nc.scalar.activation(
                            ht[:, :W], ps[:, :W],
                            mybir.ActivationFunctionType.Gelu,
                            bias=b1_t[:, e * KF + m: e * KF + m + 1])
                        hts.append(ht)
                    # phase B: y[s-tile] = prob * (hT.T @ w2), [P tok x 512 dm]
                    for n in range(ND):
                        w2ts = []
                        for k in range(KF):
                            t_ = w2p.tile([P, 512], dt, tag="w2")
                            nc.sync.dma_start(
                                t_[:], w2[e, k, :, n * 512:(n + 1) * 512])
                            w2ts.append(t_)
                        for s in range(W // P):
                            ps = psB.tile([P, 512], f32, tag="psB")
                            for k in range(KF):
                                nc.tensor.matmul(
                                    ps[:], hts[k][:, s * P:(s + 1) * P],
                                    w2ts[k][:],
                                    start=(k == 0), stop=(k == KF - 1))
                            ot = outp.tile([P, 512], f32, tag="out")
                            col = (off + c0) // P + s
                            nc.vector.tensor_scalar_mul(
                                ot[:], ps[:], pr_t[:, col:col + 1])
                            nc.sync.dma_start(
                                y[off + c0 + s * P: off + c0 + (s + 1) * P,
                                  n * 512:(n + 1) * 512], ot[:])
    nc.finalize()
    return nc


def _route(x, gate_w, gate_b):
    """Top-2 routing on host. Returns flattened (expert, prob) per routed
    pair, the by-expert sort order, per-expert counts/starts, and each
    pair's position within its expert segment."""
    T = x.shape[0]
    scores = x @ gate_w + gate_b                      # [T, E]
    part = np.argpartition(scores, -TOP_K, axis=1)[:, -TOP_K:]   # [T, 2]
    vals = np.take_along_axis(scores, part, axis=1)
    vmax = vals.max(axis=1, keepdims=True)
    ex = np.exp(vals - vmax)
    prob = ex / ex.sum(axis=1, keepdims=True)

    expert_flat = part.ravel()                        # [2T]
    prob_flat = prob.ravel().astype(np.float32)
    token_flat = np.repeat(np.arange(T), TOP_K)

    order = np.argsort(expert_flat, kind="stable")
    counts = np.bincount(expert_flat, minlength=NUM_EXPERTS)
    starts = np.zeros(NUM_EXPERTS + 1, dtype=np.int64)
    np.cumsum(counts, out=starts[1:])

    inv_order = np.empty_like(order)
    inv_order[order] = np.arange(order.size)
    pos = inv_order - starts[expert_flat]
    return (expert_flat, prob_flat, token_flat, order, counts, starts, pos)


def _prepare(x, gate_w, gate_b, w1, b1, w2, b2):
    """Host-side routing, balanced expert->(core,slot) assignment, and
    per-core input packing. Returns (in_maps, Cs, meta-for-combine)."""
    B, S, D = x.shape
    T = B * S
    xf = np.ascontiguousarray(x.reshape(T, D), dtype=np.float32)

    (expert_flat, prob_flat, token_flat, order, counts, starts, pos) = _route(
        xf, np.asarray(gate_w, np.float32), np.asarray(gate_b, np.float32))

    # balanced assignment: slot j of core c holds expert_desc[j*8 + c]
    expert_desc = np.argsort(-counts, kind="stable")
    core_of = np.empty(NUM_EXPERTS, dtype=np.int64)
    slot_of = np.empty(NUM_EXPERTS, dtype=np.int64)
    for j in range(E_LOC):
        for c in range(N_CORES):
            e = expert_desc[j * N_CORES + c]
            core_of[e] = c
            slot_of[e] = j
    Cs = []
    for j in range(E_LOC):
        mx = counts[expert_desc[j * N_CORES:(j + 1) * N_CORES]].max()
        Cs.append(max(P, int(-(-int(mx) // P)) * P))
    CT = sum(Cs)
    offs = [sum(Cs[:j]) for j in range(E_LOC)]

    xg16 = xf[token_flat[order]].astype(BF16)         # [2T, D] sorted by expert
    sorted_probs = prob_flat[order]

    w1_16 = np.asarray(w1, np.float32).astype(BF16)   # [E, D, F]
    w2_16 = np.asarray(w2, np.float32).astype(BF16)   # [E, F, D]
    b1_f = np.asarray(b1, np.float32)                 # [E, F]

    NTC = CT // P
    in_maps = []
    for c in range(N_CORES):
        xt_core = np.zeros((D, CT), dtype=BF16)
        pr_core = np.zeros((CT,), dtype=np.float32)
        w1_core = np.empty((E_LOC, D, D_FF), dtype=BF16)
        w2_core = np.empty((E_LOC, D_FF, D), dtype=BF16)
        b1_core = np.empty((E_LOC, D_FF), dtype=np.float32)
        for j in range(E_LOC):
            e = expert_desc[j * N_CORES + c]
            c_e = counts[e]
            if c_e:
                seg = slice(starts[e], starts[e] + c_e)
                xt_core[:, offs[j]:offs[j] + c_e] = xg16[seg].T
                pr_core[offs[j]:offs[j] + c_e] = sorted_probs[seg]
            w1_core[j] = w1_16[e]
            w2_core[j] = w2_16[e]
            b1_core[j] = b1_f[e]
        in_maps.append({
            "xt": np.ascontiguousarray(xt_core.reshape(KD, P, CT)),
            "w1": np.ascontiguousarray(w1_core.reshape(E_LOC, KD, P, D_FF)),
            "w2": np.ascontiguousarray(w2_core.reshape(E_LOC, KF, P, D_MODEL)),
            "pr": np.ascontiguousarray(
                pr_core.reshape(NTC, P).T),
            "b1": np.ascontiguousarray(
                b1_core.reshape(E_LOC, KF, P).transpose(2, 0, 1)
                .reshape(P, E_LOC * KF)),
        })

    meta = dict(T=T, shape=x.shape, CT=CT, offs=offs,
                core_of=core_of, slot_of=slot_of,
                expert_flat=expert_flat, prob_flat=prob_flat,
                token_flat=token_flat, pos=pos, b2=np.asarray(b2, np.float32))
    return in_maps, Cs, meta


def _combine(y_per_core, meta):
    """out[t] = sum of the token's two routed expert outputs (+ b2 term)."""
    T = meta["T"]
    CT = meta["CT"]
    offs = np.asarray(meta["offs"], dtype=np.int64)
    expert_flat = meta["expert_flat"]
    y_flat = np.concatenate(y_per_core, axis=0)       # [8*CT, D]

    rows = (meta["core_of"][expert_flat] * CT
            + offs[meta["slot_of"][expert_flat]] + meta["pos"])
    rows = rows.reshape(T, TOP_K)
    out = y_flat[rows[:, 0]] + y_flat[rows[:, 1]]

    b2_f = meta["b2"]
    if np.any(b2_f):
        combine = np.zeros((T, NUM_EXPERTS), dtype=np.float32)
        np.add.at(combine, (meta["token_flat"], expert_flat), meta["prob_flat"])
        out += combine @ b2_f
    return np.ascontiguousarray(out.reshape(meta["shape"]), dtype=np.float32)


def kernel(x, gate_w, gate_b, w1, b1, w2, b2):
    from concourse import bass_utils

    in_maps, Cs, meta = _prepare(x, gate_w, gate_b, w1, b1, w2, b2)
    nc = _build(Cs)
    res = bass_utils.run_bass_kernel_spmd(nc, in_maps, core_ids=list(range(N_CORES)))
    return _combine([res.results[c]["y"] for c in range(N_CORES)], meta)


# revision 6
# speedup vs baseline: 295.2934x; 1.0055x over previous
"""MoE layer (top-2 of 24 experts, d_model=1024, d_ff=4096, T=4096 tokens)
on 8 Trainium2 NeuronCores.

Strategy (expert-parallel, host-routed):
  - Host computes the gate (x @ gate_w + gate_b), top-2 expert ids and
    softmax probs, then gathers each expert's tokens into a padded,
    transposed buffer xT[e] = [D, C_j] (capacity 128-aligned).
  - Experts are sharded 3 per core, balanced by token count: experts are
    sorted by count and dealt into 3 "slots" (slot j of every core holds
    one of the j-th-octile experts), so slot capacities C_0 >= C_1 >= C_2
    track the count distribution instead of all padding to the global max.
  - Each core runs a Bass/Tile kernel per slot expert:
      hT = gelu(w1[e].T-tiled @ xT[e] + b1[e])   (PE + ACT, bf16 in / fp32 acc)
      y  = prob * (hT.T-tiled @ w2[e])           (PE + DVE scale)
  - Host scatters the two per-token expert outputs back together
    (out[t] = y[slot0(t)] + y[slot1(t)]), adds the b2 combine term if b2
    is nonzero (it is zero in this problem's setup_inputs).

Matmuls run in bf16 with fp32 PSUM accumulation (rel err ~3e-3 vs fp32);
b1 is applied exactly on device as the ACT per-partition bias.
"""

import numpy as np
import ml_dtypes

P = 128
D_MODEL = 1024
D_FF = 4096
NUM_EXPERTS = 24
TOP_K = 2
N_CORES = 8
E_LOC = NUM_EXPERTS // N_CORES   # 3 experts per core
KD = D_MODEL // P                # 8  k-chunks over d_model
KF = D_FF // P                   # 32 k-chunks over d_ff
ND = D_MODEL // 512              # 2  512-wide output chunks
BF16 = ml_dtypes.bfloat16


def _build(Cs, repeat=1):
    """Build the per-core Bass program (SPMD: same program, per-core data).

    Cs: per-slot token capacities (each a multiple of 128).
    repeat: run the whole compute N times (timing rigs only).
    """
    import concourse.bacc as bacc
    import concourse.mybir as mybir
    from concourse.tile import TileContext

    dt = mybir.dt.bfloat16
    f32 = mybir.dt.float32
    CT = sum(Cs)
    offs = [sum(Cs[:j]) for j in range(E_LOC)]
    NTC = CT // P                # 128-token subtiles across all slots

    nc = bacc.Bacc(None, target_bir_lowering=False)
    xT = nc.dram_tensor("xt", [KD, P, CT], dt, kind="ExternalInput")
    w1 = nc.dram_tensor("w1", [E_LOC, KD, P, D_FF], dt, kind="ExternalInput")
    w2 = nc.dram_tensor("w2", [E_LOC, KF, P, D_MODEL], dt, kind="ExternalInput")
    pr = nc.dram_tensor("pr", [P, NTC], f32, kind="ExternalInput")
    b1 = nc.dram_tensor("b1", [P, E_LOC * KF], f32, kind="ExternalInput")
    y = nc.dram_tensor("y", [CT, D_MODEL], f32, kind="ExternalOutput")

    W1C = 4                      # w1 k-tiles DMA'd in 4 column chunks
    W1CW = D_FF // W1C           # 1024 columns per chunk
    MPC = W1CW // P              # 8 m-tiles per chunk

    with TileContext(nc) as tc:
        with tc.tile_pool(name="consts", bufs=1) as consts, \
             tc.tile_pool(name="xtp", bufs=E_LOC * KD) as xtp, \
             tc.tile_pool(name="w1p", bufs=(W1C * KD) + 8) as w1p, \
             tc.tile_pool(name="w2p", bufs=40) as w2p, \
             tc.tile_pool(name="htp", bufs=KF + 2) as htp, \
             tc.tile_pool(name="outp", bufs=4) as outp, \
             tc.tile_pool(name="psA", bufs=4, space="PSUM") as psA, \
             tc.tile_pool(name="psB", bufs=4, space="PSUM") as psB:

            b1_t = consts.tile([P, E_LOC * KF], f32, tag="b1")
            nc.sync.dma_start(b1_t[:], b1[:, :])
            pr_t = consts.tile([P, NTC], f32, tag="pr")
            nc.sync.dma_start(pr_t[:], pr[:, :])

            xts = {}   # (slot j, k) -> [P, Cs[j]] tile
            w1ts = {}  # (k, chunk) -> [P, 1024] tile, current expert

            def load_w1_chunk(e, c4):
                for k in range(KD):
                    t_ = w1p.tile([P, W1CW], dt, tag="w1")
                    nc.sync.dma_start(
                        t_[:], w1[e, k, :, c4 * W1CW:(c4 + 1) * W1CW])
                    w1ts[(k, c4)] = t_

            def load_xt(j):
                for k in range(KD):
                    t_ = xtp.tile([P, Cs[j]], dt, tag="xt")
                    nc.sync.dma_start(
                        t_[:], xT[k, :, offs[j]:offs[j] + Cs[j]])
                    xts[(j, k)] = t_

            first = True
            for e in [e for _ in range(repeat) for e in range(E_LOC)]:
                C = Cs[e]
                off = offs[e]
                load_w1_chunk(e, 0)
                if first:
                    # slot-0 xt first (phase A of expert 0 needs it), then the
                    # other slots prefetch behind it
                    for j in range(E_LOC):
                        load_xt(j)
                    first = False
                for c4 in range(1, W1C):
                    load_w1_chunk(e, c4)

                NT = (C + 511) // 512
                for t in range(NT):
                    c0 = t * 512
                    W = min(512, C - c0)
                    # phase A: hT[m] = gelu(w1k.T @ xT + b1), [P dff x W tok]
                    hts = []
                    for m in range(KF):
                        ps = psA.tile([P, 512], f32, tag="psA")
                        for k in range(KD):
                            nc.tensor.matmul(
                                ps[:, :W],
                                w1ts[(k, m // MPC)][:, (m % MPC) * P:
                                                    (m % MPC + 1) * P],
                                xts[(e, k)][:, c0:c0 + W],
                                start=(k == 0), stop=(k == KD - 1))
                        ht = htp.tile([P, 512], dt, tag="ht")
                        nc.scalar.activation(
                            ht[:, :W], ps[:, :W],
                            mybir.ActivationFunctionType.Gelu,
                            bias=b1_t[:, e * KF + m: e * KF + m + 1])
                        hts.append(ht)
                    # phase B: y[s-tile] = prob * (hT.T @ w2), [P tok x 512 dm]
                    for n in range(ND):
                        w2ts = []
                        for k in range(KF):
                            t_ = w2p.tile([P, 512], dt, tag="w2")
                            nc.sync.dma_start(
                                t_[:], w2[e, k, :, n * 512:(n + 1) * 512])
                            w2ts.append(t_)
                        for s in range(W // P):
                            ps = psB.tile([P, 512], f32, tag="psB")
                            for k in range(KF):
                                nc.tensor.matmul(
                                    ps[:], hts[k][:, s * P:(s + 1) * P],
                                    w2ts[k][:],
                                    start=(k == 0), stop=(k == KF - 1))
                            ot = outp.tile([P, 512], f32, tag="out")
                            col = (off + c0) // P + s
                            nc.vector.tensor_scalar_mul(
                                ot[:], ps[:], pr_t[:, col:col + 1])
                            nc.sync.dma_start(
                                y[off + c0 + s * P: off + c0 + (s + 1) * P,
                                  n * 512:(n + 1) * 512], ot[:])
    nc.finalize()
    return nc


def _route(x, gate_w, gate_b):
    """Top-2 routing on host. Returns flattened (expert, prob) per routed
    pair, the by-expert sort order, per-expert counts/starts, and each
    pair's position within its expert segment."""
    T = x.shape[0]
    scores = x @ gate_w + gate_b                      # [T, E]
    part = np.argpartition(scores, -TOP_K, axis=1)[:, -TOP_K:]   # [T, 2]
    vals = np.take_along_axis(scores, part, axis=1)
    vmax = vals.max(axis=1, keepdims=True)
    ex = np.exp(vals - vmax)
    prob = ex / ex.sum(axis=1, keepdims=True)

    expert_flat = part.ravel()                        # [2T]
    prob_flat = prob.ravel().astype(np.float32)
    token_flat = np.repeat(np.arange(T), TOP_K)

    order = np.argsort(expert_flat, kind="stable")
    counts = np.bincount(expert_flat, minlength=NUM_EXPERTS)
    starts = np.zeros(NUM_EXPERTS + 1, dtype=np.int64)
    np.cumsum(counts, out=starts[1:])

    inv_order = np.empty_like(order)
    inv_order[order] = np.arange(order.size)
    pos = inv_order - starts[expert_flat]
    return (expert_flat, prob_flat, token_flat, order, counts, starts, pos)


def _prepare(x, gate_w, gate_b, w1, b1, w2, b2):
    """Host-side routing, balanced expert->(core,slot) assignment, and
    per-core input packing. Returns (in_maps, Cs, meta-for-combine)."""
    B, S, D = x.shape
    T = B * S
    xf = np.ascontiguousarray(x.reshape(T, D), dtype=np.float32)

    (expert_flat, prob_flat, token_flat, order, counts, starts, pos) = _route(
        xf, np.asarray(gate_w, np.float32), np.asarray(gate_b, np.float32))

    # balanced assignment: slot j of core c holds expert_desc[j*8 + c]
    expert_desc = np.argsort(-counts, kind="stable")
    core_of = np.empty(NUM_EXPERTS, dtype=np.int64)
    slot_of = np.empty(NUM_EXPERTS, dtype=np.int64)
    for j in range(E_LOC):
        for c in range(N_CORES):
            e = expert_desc[j * N_CORES + c]
            core_of[e] = c
            slot_of[e] = j
    Cs = []
    for j in range(E_LOC):
        mx = counts[expert_desc[j * N_CORES:(j + 1) * N_CORES]].max()
        Cs.append(max(P, int(-(-int(mx) // P)) * P))
    CT = sum(Cs)
    offs = [sum(Cs[:j]) for j in range(E_LOC)]

    xg16 = xf[token_flat[order]].astype(BF16)         # [2T, D] sorted by expert
    sorted_probs = prob_flat[order]

    w1_16 = np.asarray(w1, np.float32).astype(BF16)   # [E, D, F]
    w2_16 = np.asarray(w2, np.float32).astype(BF16)   # [E, F, D]
    b1_f = np.asarray(b1, np.float32)                 # [E, F]

    NTC = CT // P
    in_maps = []
    for c in range(N_CORES):
        xt_core = np.zeros((D, CT), dtype=BF16)
        pr_core = np.zeros((CT,), dtype=np.float32)
        w1_core = np.empty((E_LOC, D, D_FF), dtype=BF16)
        w2_core = np.empty((E_LOC, D_FF, D), dtype=BF16)
        b1_core = np.empty((E_LOC, D_FF), dtype=np.float32)
        for j in range(E_LOC):
            e = expert_desc[j * N_CORES + c]
            c_e = counts[e]
            if c_e:
                seg = slice(starts[e], starts[e] + c_e)
                xt_core[:, offs[j]:offs[j] + c_e] = xg16[seg].T
                pr_core[offs[j]:offs[j] + c_e] = sorted_probs[seg]
            w1_core[j] = w1_16[e]
            w2_core[j] = w2_16[e]
            b1_core[j] = b1_f[e]
        in_maps.append({
            "xt": np.ascontiguousarray(xt_core.reshape(KD, P, CT)),
            "w1": np.ascontiguousarray(w1_core.reshape(E_LOC, KD, P, D_FF)),
            "w2": np.ascontiguousarray(w2_core.reshape(E_LOC, KF, P, D_MODEL)),
            "pr": np.ascontiguousarray(
                pr_core.reshape(NTC, P).T),
            "b1": np.ascontiguousarray(
                b1_core.reshape(E_LOC, KF, P).transpose(2, 0, 1)
                .reshape(P, E_LOC * KF)),
        })

    meta = dict(T=T, shape=x.shape, CT=CT, offs=offs,
                core_of=core_of, slot_of=slot_of,
                expert_flat=expert_flat, prob_flat=prob_flat,
                token_flat=token_flat, pos=pos, b2=np.asarray(b2, np.float32))
    return in_maps, Cs, meta


def _combine(y_per_core, meta):
    """out[t] = sum of the token's two routed expert outputs (+ b2 term)."""
    T = meta["T"]
    CT = meta["CT"]
    offs = np.asarray(meta["offs"], dtype=np.int64)
    expert_flat = meta["expert_flat"]
    y_flat = np.concatenate(y_per_core, axis=0)       # [8*CT, D]

    rows = (meta["core_of"][expert_flat] * CT
            + offs[meta["slot_of"][expert_flat]] + meta["pos"])
    rows = rows.reshape(T, TOP_K)
    out = y_flat[rows[:, 0]] + y_flat[rows[:, 1]]

    b2_f = meta["b2"]
    if np.any(b2_f):
        combine = np.zeros((T, NUM_EXPERTS), dtype=np.float32)
        np.add.at(combine, (meta["token_flat"], expert_flat), meta["prob_flat"])
        out += combine @ b2_f
    return np.ascontiguousarray(out.reshape(meta["shape"]), dtype=np.float32)


def kernel(x, gate_w, gate_b, w1, b1, w2, b2):
    from concourse import bass_utils

    in_maps, Cs, meta = _prepare(x, gate_w, gate_b, w1, b1, w2, b2)
    nc = _build(Cs)
    res = bass_utils.run_bass_kernel_spmd(nc, in_maps, core_ids=list(range(N_CORES)))
    return _combine([res.results[c]["y"] for c in range(N_CORES)], meta)
